# revision 5
# baseline (speedup 1.0000x reference)
"""Trainium2 Bass kernel for nn_Attn_19464791785826.

Reference computation (per batch b of 32):
    proj[l, :] = enc[b, l] @ W.T + bias            # [4096, 512]
    energies[l] = hidden[b] . proj[l]              # [4096]
    out[b, 0, :] = softmax(energies)               # [4096]

Algebraic rewrite: energies[l] = (hidden[b] @ W) . enc[b, l] + hidden[b].bias.
The bias term is constant across l, so softmax cancels it exactly. q = hidden@W
is a tiny [32, 512] matrix computed on the host; the device does the
memory-bound part: a mat-vec against the encoder_outputs tensor + softmax.

Precision: enc and q are converted to fp16 on the host. This halves HBM
traffic (the kernel is memory-bound) and doubles DVE throughput. Products are
accumulated in fp32 (tensor_tensor_reduce accum), so the energy error is
~1e-3 relative, far inside the 2e-2 gate.

Sharding: data-parallel over batch. 32 batches / 8 cores = 4 per core.
No collectives; the host gathers per-core [4, 4096] outputs and undoes the
on-chip layout permutation.

Per-core dataflow:
  - all enc chunk DMAs are issued first so HBM is saturated from t=0;
    each chunk lands as [128, tpc, H] fp16 with 8 KiB contiguous descriptors
    (SBUF partition p holds l = c*CL + p*tpc + i).
  - q[b] (DMA'd as [bpc, H] fp16) is partition-replicated once per batch via
    a rank-1 PE matmul (ones ⊗ q).
  - per l-subtile, ONE fused DVE tensor_tensor_reduce computes
    junk = et * q and eb[:, col] = sum_h(junk) with fp32 accumulation --
    a single pass over the data on one engine (2-byte operands enable the
    DVE fast path).
  - softmax per batch over the [128, ncols] energy tile: free-dim max, PE
    transpose + reduce for the cross-partition max, ScalarE exp with fused
    per-partition sum, ones-matmul for the cross-partition sum, reciprocal,
    PE transpose to [ncols, 128] with normalization fused into the
    PSUM->SBUF evacuation, contiguous DMA out.
"""

import numpy as np

import concourse.bass as bass
from concourse import bacc
import concourse.mybir as mybir
import concourse.tile as tile
from concourse.bass_utils import run_bass_kernel_spmd
from concourse.masks import make_identity

H = 512
L = 4096
B = 32
N_CORES = 8
BPC = B // N_CORES  # batches per core
CHUNK_L = 1024

F32 = mybir.dt.float32
F16 = mybir.dt.float16


def emit_core_kernel(nc, tc, enc, q, out, bpc, l_total, chunk_l):
    """Emit the per-core kernel into an open TileContext."""
    n_chunks = l_total // chunk_l
    tpc = chunk_l // 128          # l-subtiles per chunk
    ncols = l_total // 128        # energy columns per batch

    import contextlib
    ctx = contextlib.ExitStack()
    with ctx:
        const = ctx.enter_context(tc.tile_pool(name="const", bufs=1))
        qp = ctx.enter_context(tc.tile_pool(name="qp", bufs=1))
        encp = ctx.enter_context(tc.tile_pool(name="encp", bufs=6))
        junkp = ctx.enter_context(tc.tile_pool(name="junkp", bufs=2))
        epool = ctx.enter_context(tc.tile_pool(name="epool", bufs=2))
        small = ctx.enter_context(tc.tile_pool(name="small", bufs=2))
        opool = ctx.enter_context(tc.tile_pool(name="opool", bufs=2))
        psq = ctx.enter_context(tc.tile_pool(name="psq", bufs=2, space="PSUM"))
        ptp = ctx.enter_context(tc.tile_pool(name="ptp", bufs=2, space="PSUM"))
        pss = ctx.enter_context(tc.tile_pool(name="pss", bufs=4, space="PSUM"))

        # ---- input DMAs up front: HBM busy from t=0 --------------------
        q_sb = qp.tile([1, bpc * H], F16)  # all 4 q vectors on partition 0
        nc.sync.dma_start(out=q_sb, in_=q[:, :])
        et = {}
        for b in range(bpc):
            for c in range(n_chunks):
                t = encp.tile([128, tpc, H], F16, tag="enc")
                nc.sync.dma_start(
                    out=t,
                    in_=enc[b, c * chunk_l:(c + 1) * chunk_l, :]
                        .rearrange("(p i) h -> p i h", p=128),
                )
                et[(b, c)] = t

        # ---- constants -------------------------------------------------
        ident = const.tile([128, 128], F32)
        make_identity(nc, ident)
        ones_row16 = const.tile([1, 128], F16)
        nc.vector.memset(ones_row16, 1.0)
        ones_row = const.tile([1, 128], F32)
        nc.vector.memset(ones_row, 1.0)
        neg_ones_row = const.tile([1, 128], F32)
        nc.vector.memset(neg_ones_row, -1.0)
        ones_col = const.tile([128, 1], F32)
        nc.vector.memset(ones_col, 1.0)

        # preload the Exp table so batch 0's softmax doesn't stall on it
        dexp = small.tile([1, 1], F32, tag="dexp")
        nc.scalar.activation(dexp, ones_row[:1, :1],
                             mybir.ActivationFunctionType.Exp)

        # ---- q[b] partition-replicated via rank-1 PE matmul ------------
        q_rep = qp.tile([128, bpc, H], F16)
        for b in range(bpc):
            qb_ps = psq.tile([128, H], F32, tag="qb")
            nc.tensor.matmul(qb_ps, lhsT=ones_row16,
                             rhs=q_sb[:, b * H:(b + 1) * H],
                             start=True, stop=True)
            nc.scalar.copy(q_rep[:, b, :], qb_ps)

        # ---- main loop: one fused mult+reduce per l-subtile ------------
        for b in range(bpc):
            eb = epool.tile([128, ncols], F32)  # eb[p, c*tpc+i] = E[c*CL + p*tpc + i]
            for c in range(n_chunks):
                t = et[(b, c)]
                for i in range(tpc):
                    col = c * tpc + i
                    junkt = junkp.tile([128, H], F16, tag="junk")
                    nc.vector.tensor_mul(junkt, t[:, i, :], q_rep[:, b, :])
                    nc.vector.tensor_reduce(
                        eb[:, col:col + 1], junkt,
                        axis=mybir.AxisListType.X, op=mybir.AluOpType.add)

            # ---- softmax over the [128, ncols] energy tile -------------
            mp = small.tile([128, 1], F32)
            nc.vector.tensor_reduce(mp, eb, axis=mybir.AxisListType.X,
                                    op=mybir.AluOpType.max)
            mt_ps = pss.tile([1, 128], F32, tag="sp")
            nc.tensor.transpose(mt_ps, mp, ident)
            mt = small.tile([1, 128], F32)
            nc.scalar.copy(mt, mt_ps)
            mg = small.tile([1, 1], F32)
            nc.vector.tensor_reduce(mg, mt, axis=mybir.AxisListType.X,
                                    op=mybir.AluOpType.max)
            # broadcast -max to all partitions
            nm_ps = pss.tile([128, 1], F32, tag="sp")
            nc.tensor.matmul(nm_ps, lhsT=neg_ones_row, rhs=mg,
                             start=True, stop=True)
            negmax = small.tile([128, 1], F32)
            nc.scalar.copy(negmax, nm_ps)
            # exp(e - max) with fused per-partition sum
            pb = epool.tile([128, ncols], F32, tag="pb")
            sp_t = small.tile([128, 1], F32)
            nc.scalar.activation(pb, eb, mybir.ActivationFunctionType.Exp,
                                 bias=negmax, scale=1.0, accum_out=sp_t)
            # cross-partition sum -> total, then 1/total broadcast
            tot_ps = pss.tile([1, 1], F32, tag="sp")
            nc.tensor.matmul(tot_ps, lhsT=sp_t, rhs=ones_col,
                             start=True, stop=True)
            rec = small.tile([1, 1], F32)
            nc.vector.reciprocal(rec, tot_ps)
            rb_ps = pss.tile([128, 1], F32, tag="sp")
            nc.tensor.matmul(rb_ps, lhsT=ones_row, rhs=rec,
                             start=True, stop=True)
            rbc = small.tile([128, 1], F32)
            nc.scalar.copy(rbc, rb_ps)
            # transpose to [ncols, 128]; normalize on the PSUM->SBUF copy
            pt_ps = ptp.tile([ncols, 128], F32, tag="pt")
            nc.tensor.transpose(pt_ps, pb, ident)
            ob = opool.tile([ncols, 128], F32)
            nc.vector.tensor_scalar_mul(ob, pt_ps, rbc[:ncols, :])
            nc.sync.dma_start(out=out[b].rearrange("(t p) -> t p", p=128),
                              in_=ob)


def unpermute(out2d, l_total=L, chunk_l=CHUNK_L):
    """Undo the on-chip l-layout: device out[b, (c*tpc+i)*128 + p] holds
    prob(l = c*chunk_l + p*tpc + i)."""
    nb = out2d.shape[0]
    n_chunks = l_total // chunk_l
    tpc = chunk_l // 128
    return (out2d.reshape(nb, n_chunks, tpc, 128)
                 .transpose(0, 1, 3, 2)
                 .reshape(nb, l_total))


def build_bass(bpc=BPC, l_total=L, chunk_l=CHUNK_L):
    nc = bacc.Bacc(None)
    enc = nc.declare_dram_parameter("enc", [bpc, l_total, H], F16, isOutput=False)
    q = nc.declare_dram_parameter("q", [bpc, H], F16, isOutput=False)
    out = nc.declare_dram_parameter("out", [bpc, l_total], F32, isOutput=True)
    with tile.TileContext(nc) as tc:
        emit_core_kernel(nc, tc, enc, q, out, bpc, l_total, chunk_l)
    nc.compile()
    return nc


_NC_CACHE = {}


def make_in_maps(hidden, encoder_outputs, W):
    """Host-side prep: q = hidden @ W, fp16 conversion, batch sharding."""
    q = (np.asarray(hidden, dtype=np.float32)[0]
         @ np.asarray(W, dtype=np.float32)).astype(np.float16)      # [B, H]
    enc16 = np.asarray(encoder_outputs).astype(np.float16)          # [B, L, H]
    in_maps = []
    for c in range(N_CORES):
        sl = slice(c * BPC, (c + 1) * BPC)
        in_maps.append({
            "enc": np.ascontiguousarray(enc16[sl]),
            "q": np.ascontiguousarray(q[sl]),
        })
    return in_maps


def kernel(hidden, encoder_outputs, W, b):
    # b only shifts every energy in a batch by a constant; softmax cancels it.
    key = "full"
    if key not in _NC_CACHE:
        _NC_CACHE[key] = build_bass()
    nc = _NC_CACHE[key]

    in_maps = make_in_maps(hidden, encoder_outputs, W)
    results = run_bass_kernel_spmd(nc, in_maps, list(range(N_CORES))).results
    out = np.concatenate([r["out"] for r in results], axis=0)  # [32, 4096]
    out = unpermute(out)
    return out[:, None, :].astype(np.float32)


# revision 6
# speedup vs baseline: 1.4007x; 1.4007x over previous
"""Trainium2 Bass kernel for nn_Attn_19464791785826.

Reference computation (per batch b of 32):
    proj[l, :] = enc[b, l] @ W.T + bias            # [4096, 512]
    energies[l] = hidden[b] . proj[l]              # [4096]
    out[b, 0, :] = softmax(energies)               # [4096]

Algebraic rewrite: energies[l] = (hidden[b] @ W) . enc[b, l] + hidden[b].bias.
The bias term is constant across l, so softmax cancels it exactly. q = hidden@W
is a tiny [32, 512] matrix computed on the host; the device does the
memory-bound part: a mat-vec against the encoder_outputs tensor + softmax.

Precision: enc and q are converted to fp16 on the host. This halves HBM
traffic (the kernel is memory-bound) and doubles DVE throughput. Products are
accumulated in fp32 (tensor_tensor_reduce accum), so the energy error is
~1e-3 relative, far inside the 2e-2 gate.

Sharding: data-parallel over batch. 32 batches / 8 cores = 4 per core.
No collectives; the host gathers per-core [4, 4096] outputs and undoes the
on-chip layout permutation.

Per-core dataflow:
  - all enc chunk DMAs are issued first so HBM is saturated from t=0;
    each chunk lands as [128, tpc, H] fp16 with 8 KiB contiguous descriptors
    (SBUF partition p holds l = c*CL + p*tpc + i).
  - q[b] (DMA'd as [bpc, H] fp16) is partition-replicated once per batch via
    a rank-1 PE matmul (ones ⊗ q).
  - per l-subtile, ONE fused DVE tensor_tensor_reduce computes
    junk = et * q and eb[:, col] = sum_h(junk) with fp32 accumulation --
    a single pass over the data on one engine (2-byte operands enable the
    DVE fast path).
  - softmax per batch over the [128, ncols] energy tile: free-dim max, PE
    transpose + reduce for the cross-partition max, ScalarE exp with fused
    per-partition sum, ones-matmul for the cross-partition sum, reciprocal,
    PE transpose to [ncols, 128] with normalization fused into the
    PSUM->SBUF evacuation, contiguous DMA out.
"""

import numpy as np

import concourse.bass as bass
from concourse import bacc
import concourse.mybir as mybir
import concourse.tile as tile
from concourse.bass_utils import run_bass_kernel_spmd
from concourse.masks import make_identity

H = 512
L = 4096
B = 32
N_CORES = 8
BPC = B // N_CORES  # batches per core
CHUNK_L = 1024

F32 = mybir.dt.float32
F16 = mybir.dt.float16


def emit_core_kernel(nc, tc, enc, q, out, bpc, l_total, chunk_l):
    """Emit the per-core kernel into an open TileContext."""
    n_chunks = l_total // chunk_l
    tpc = chunk_l // 128          # l-subtiles per chunk
    ncols = l_total // 128        # energy columns per batch

    import contextlib
    ctx = contextlib.ExitStack()
    with ctx:
        const = ctx.enter_context(tc.tile_pool(name="const", bufs=1))
        qp = ctx.enter_context(tc.tile_pool(name="qp", bufs=1))
        encp = ctx.enter_context(tc.tile_pool(name="encp", bufs=6))
        junkp = ctx.enter_context(tc.tile_pool(name="junkp", bufs=2))
        epool = ctx.enter_context(tc.tile_pool(name="epool", bufs=2))
        small = ctx.enter_context(tc.tile_pool(name="small", bufs=2))
        opool = ctx.enter_context(tc.tile_pool(name="opool", bufs=2))
        psq = ctx.enter_context(tc.tile_pool(name="psq", bufs=2, space="PSUM"))
        ptp = ctx.enter_context(tc.tile_pool(name="ptp", bufs=2, space="PSUM"))
        pss = ctx.enter_context(tc.tile_pool(name="pss", bufs=4, space="PSUM"))

        # ---- input DMAs up front: HBM busy from t=0 --------------------
        q_sb = qp.tile([1, bpc * H], F16)  # all 4 q vectors on partition 0
        nc.sync.dma_start(out=q_sb, in_=q[:, :])
        et = {}
        for b in range(bpc):
            for c in range(n_chunks):
                t = encp.tile([128, tpc, H], F16, tag="enc")
                nc.sync.dma_start(
                    out=t,
                    in_=enc[b, c * chunk_l:(c + 1) * chunk_l, :]
                        .rearrange("(p i) h -> p i h", p=128),
                )
                et[(b, c)] = t

        # ---- constants -------------------------------------------------
        ident = const.tile([128, 128], F32)
        make_identity(nc, ident)
        ones_row16 = const.tile([1, 128], F16)
        nc.vector.memset(ones_row16, 1.0)
        ones_row = const.tile([1, 128], F32)
        nc.vector.memset(ones_row, 1.0)
        neg_ones_row = const.tile([1, 128], F32)
        nc.vector.memset(neg_ones_row, -1.0)
        ones_col = const.tile([128, 1], F32)
        nc.vector.memset(ones_col, 1.0)

        # preload the Exp table so batch 0's softmax doesn't stall on it
        dexp = small.tile([1, 1], F32, tag="dexp")
        nc.scalar.activation(dexp, ones_row[:1, :1],
                             mybir.ActivationFunctionType.Exp)

        # ---- q[b] partition-replicated via rank-1 PE matmul ------------
        q_rep = qp.tile([128, bpc, H], F16)
        for b in range(bpc):
            qb_ps = psq.tile([128, H], F32, tag="qb")
            nc.tensor.matmul(qb_ps, lhsT=ones_row16,
                             rhs=q_sb[:, b * H:(b + 1) * H],
                             start=True, stop=True)
            nc.scalar.copy(q_rep[:, b, :], qb_ps)

        # ---- main loop: one fused mult+reduce per l-subtile ------------
        for b in range(bpc):
            eb = epool.tile([128, ncols], F32)  # eb[p, c*tpc+i] = E[c*CL + p*tpc + i]
            for c in range(n_chunks):
                t = et[(b, c)]
                for i in range(tpc):
                    col = c * tpc + i
                    junkt = junkp.tile([128, H], F16, tag="junk")
                    nc.vector.scalar_tensor_tensor(
                        junkt, t[:, i, :], 1.0, q_rep[:, b, :],
                        mybir.AluOpType.mult, mybir.AluOpType.mult,
                        accum_out=eb[:, col:col + 1])

            # ---- softmax over the [128, ncols] energy tile -------------
            mp = small.tile([128, 1], F32)
            nc.vector.tensor_reduce(mp, eb, axis=mybir.AxisListType.X,
                                    op=mybir.AluOpType.max)
            mt_ps = pss.tile([1, 128], F32, tag="sp")
            nc.tensor.transpose(mt_ps, mp, ident)
            mt = small.tile([1, 128], F32)
            nc.scalar.copy(mt, mt_ps)
            mg = small.tile([1, 1], F32)
            nc.vector.tensor_reduce(mg, mt, axis=mybir.AxisListType.X,
                                    op=mybir.AluOpType.max)
            # broadcast -max to all partitions
            nm_ps = pss.tile([128, 1], F32, tag="sp")
            nc.tensor.matmul(nm_ps, lhsT=neg_ones_row, rhs=mg,
                             start=True, stop=True)
            negmax = small.tile([128, 1], F32)
            nc.scalar.copy(negmax, nm_ps)
            # exp(e - max) with fused per-partition sum
            pb = epool.tile([128, ncols], F32, tag="pb")
            sp_t = small.tile([128, 1], F32)
            nc.scalar.activation(pb, eb, mybir.ActivationFunctionType.Exp,
                                 bias=negmax, scale=1.0, accum_out=sp_t)
            # cross-partition sum -> total, then 1/total broadcast
            tot_ps = pss.tile([1, 1], F32, tag="sp")
            nc.tensor.matmul(tot_ps, lhsT=sp_t, rhs=ones_col,
                             start=True, stop=True)
            rec = small.tile([1, 1], F32)
            nc.vector.reciprocal(rec, tot_ps)
            rb_ps = pss.tile([128, 1], F32, tag="sp")
            nc.tensor.matmul(rb_ps, lhsT=ones_row, rhs=rec,
                             start=True, stop=True)
            rbc = small.tile([128, 1], F32)
            nc.scalar.copy(rbc, rb_ps)
            # transpose to [ncols, 128]; normalize on the PSUM->SBUF copy
            pt_ps = ptp.tile([ncols, 128], F32, tag="pt")
            nc.tensor.transpose(pt_ps, pb, ident)
            ob = opool.tile([ncols, 128], F32)
            nc.vector.tensor_scalar_mul(ob, pt_ps, rbc[:ncols, :])
            nc.sync.dma_start(out=out[b].rearrange("(t p) -> t p", p=128),
                              in_=ob)


def unpermute(out2d, l_total=L, chunk_l=CHUNK_L):
    """Undo the on-chip l-layout: device out[b, (c*tpc+i)*128 + p] holds
    prob(l = c*chunk_l + p*tpc + i)."""
    nb = out2d.shape[0]
    n_chunks = l_total // chunk_l
    tpc = chunk_l // 128
    return (out2d.reshape(nb, n_chunks, tpc, 128)
                 .transpose(0, 1, 3, 2)
                 .reshape(nb, l_total))


def build_bass(bpc=BPC, l_total=L, chunk_l=CHUNK_L):
    nc = bacc.Bacc(None)
    enc = nc.declare_dram_parameter("enc", [bpc, l_total, H], F16, isOutput=False)
    q = nc.declare_dram_parameter("q", [bpc, H], F16, isOutput=False)
    out = nc.declare_dram_parameter("out", [bpc, l_total], F32, isOutput=True)
    with tile.TileContext(nc) as tc:
        emit_core_kernel(nc, tc, enc, q, out, bpc, l_total, chunk_l)
    nc.compile()
    return nc


_NC_CACHE = {}


def make_in_maps(hidden, encoder_outputs, W):
    """Host-side prep: q = hidden @ W, fp16 conversion, batch sharding."""
    q = (np.asarray(hidden, dtype=np.float32)[0]
         @ np.asarray(W, dtype=np.float32)).astype(np.float16)      # [B, H]
    enc16 = np.asarray(encoder_outputs).astype(np.float16)          # [B, L, H]
    in_maps = []
    for c in range(N_CORES):
        sl = slice(c * BPC, (c + 1) * BPC)
        in_maps.append({
            "enc": np.ascontiguousarray(enc16[sl]),
            "q": np.ascontiguousarray(q[sl]),
        })
    return in_maps


def kernel(hidden, encoder_outputs, W, b):
    # b only shifts every energy in a batch by a constant; softmax cancels it.
    key = "full"
    if key not in _NC_CACHE:
        _NC_CACHE[key] = build_bass()
    nc = _NC_CACHE[key]

    in_maps = make_in_maps(hidden, encoder_outputs, W)
    results = run_bass_kernel_spmd(nc, in_maps, list(range(N_CORES))).results
    out = np.concatenate([r["out"] for r in results], axis=0)  # [32, 4096]
    out = unpermute(out)
    return out[:, None, :].astype(np.float32)
